# revision 1
# baseline (speedup 1.0000x reference)
"""Multi-head attention (B=2, S=2048, D=768, H=12) on 8 Trainium2 cores.

Sharding: core c -> batch b = c // 4, head-group g = c % 4 (3 heads of 12).
Each core gets its batch's activations pre-transposed on the host (x^T
[768, 2048] fp32 — a pure layout permutation, part of shard prep) plus its
head-group's weight shards.  Each core computes Q/K/V projections for its
head group, attention, and a partial output (its head rows of Wo).  The
host sums the 4 partials per batch and adds bo.

Device kernel (per core):
  - x^T loaded with a casting DMA (fp32 -> bf16, 8KB/partition contiguous
    descriptors) straight into the projection rhs layout.
  - Q^T, K^T per head as [64, 2048] bf16 tiles (lhsT = W chunks); V natural
    [2048, 3*65] bf16 with a ones column per head (the softmax denominator
    rides the PV matmul).
  - scores computed transposed: S^T[k, q] = K Q^T on PE; exp on ScalarE
    (scale = 1/sqrt(64), PSUM->SBUF bf16); PV matmul V_aug^T @ P^T
    accumulates O^T[65, q] in PSUM, row 64 = denominator.  q is processed
    in halves of 1024, heads 0/1 interleaved in the k loop, and both
    scores emitted before either PV so the in-order PE queue never blocks
    on the ACT exp (keeps PE streaming and HAM at K=8/8).
  - normalize O^T with approx-reciprocal + partition-broadcast off the
    critical path; Wo row-shard matmuls (interleaved into the solo head's
    k loop) produce the partial [2048, 768] fp32 output.
"""

import sys

for _p in ("/opt/trn_rl_repo",):
    if _p not in sys.path:
        sys.path.append(_p)

import numpy as np

B = 2
S = 2048
D = 768
H = 12
DK = 64
HG = 3            # heads per core
HD = HG * DK      # 192
P = 128
NS = S // P       # 16 s-tiles
ND = D // P       # 6 d-chunks
NB = S // 512     # 4 s-blocks
QH = 1024         # q half

_CACHE = {}


def _build_nc(use_bias_qkv):
    import concourse.bacc as bacc
    import concourse.tile as tile
    from concourse import mybir
    from contextlib import ExitStack

    BF = mybir.dt.bfloat16
    F32 = mybir.dt.float32
    EXP = mybir.ActivationFunctionType.Exp

    nc = bacc.Bacc("TRN2", target_bir_lowering=False, debug=False)

    xqT = nc.dram_tensor("xqT", [D, S], F32, kind="ExternalInput").ap()
    xkT = nc.dram_tensor("xkT", [D, S], F32, kind="ExternalInput").ap()
    xvT = nc.dram_tensor("xvT", [D, S], F32, kind="ExternalInput").ap()
    wq = nc.dram_tensor("wq", [D, HD], F32, kind="ExternalInput").ap()
    wk = nc.dram_tensor("wk", [D, HD], F32, kind="ExternalInput").ap()
    wv = nc.dram_tensor("wv", [D, HD], F32, kind="ExternalInput").ap()
    wo = nc.dram_tensor("wo", [HD, D], F32, kind="ExternalInput").ap()
    bqkv = nc.dram_tensor("bqkv", [3, HD], F32, kind="ExternalInput").ap()
    y = nc.dram_tensor("y", [S, D], F32, kind="ExternalOutput").ap()

    with tile.TileContext(nc) as tc, ExitStack() as ctx:
        wpool = ctx.enter_context(tc.tile_pool(name="weights", bufs=1))
        apool = ctx.enter_context(tc.tile_pool(name="acts", bufs=1))

        QTa = apool.tile([P, S], BF, tag="qta")    # heads 0,1 stacked on partitions
        QTb = apool.tile([DK, S], BF, tag="qtb")   # head 2
        KTa = apool.tile([P, S], BF, tag="kta")
        KTb = apool.tile([DK, S], BF, tag="ktb")
        QT = [QTa[0:DK, :], QTa[DK:P, :], QTb[:, :]]
        KT = [KTa[0:DK, :], KTa[DK:P, :], KTb[:, :]]
        V = apool.tile([P, NS, 3 * 65], BF, tag="v")
        OC1 = apool.tile([P, S], BF, tag="oc1")    # heads 0,1 of O^T (normalized)
        OC2 = apool.tile([DK, S], BF, tag="oc2")   # head 2

        # ================= phase 1: load x^T + projections =================
        # inputs loaded v, k, q as per-d-chunk casting DMAs so projections
        # chase the loads; Q/K projections run d-outer over s-block pairs so
        # the stationary weight is reused across streams (few LDWEIGHTS).
        with tc.tile_pool(name="xt", bufs=2) as xt_pool, \
             tc.tile_pool(name="mm_ps", bufs=2, space="PSUM") as mm_pool, \
             tc.tile_pool(name="qka_ps", bufs=1, space="PSUM") as qka_pool, \
             tc.tile_pool(name="qkb_ps", bufs=1, space="PSUM") as qkb_pool:

            # x^T loads: one whole-tile casting DMA per (input, d-chunk) so
            # downstream matmuls chase individual chunk arrivals
            xtc = {}
            for name, xT in (("wv", xvT), ("wk", xkT), ("wq", xqT)):
                for dc in range(ND):
                    t = xt_pool.tile([P, S], BF, tag=f"xt{dc}", name=f"xt_{name}{dc}")
                    nc.gpsimd.dma_start(out=t, in_=xT[dc * P : (dc + 1) * P, :])
                    xtc[(name, dc)] = t

            # weights (HWDGE queue, parallel with the gpsimd loads)
            w_bf = {}
            for name, w in (("wv", wv), ("wk", wk), ("wq", wq)):
                wf = wpool.tile([P, ND, HD], F32, tag=f"{name}_f32", name=f"{name}_f32")
                nc.sync.dma_start(out=wf, in_=w.rearrange("(nd p) h -> p nd h", p=P))
                wb = wpool.tile([P, ND, HD], BF, tag=f"{name}_bf", name=f"{name}_bf")
                nc.any.tensor_copy(out=wb, in_=wf)
                w_bf[name] = wb
            wo_f1 = wpool.tile([P, D], F32, tag="wo_f1")
            nc.sync.dma_start(out=wo_f1, in_=wo[0:P, :])
            wo_f2 = wpool.tile([DK, D], F32, tag="wo_f2")
            nc.sync.dma_start(out=wo_f2, in_=wo[P:HD, :])
            wo_b1 = wpool.tile([P, D], BF, tag="wo_b1")
            nc.any.tensor_copy(out=wo_b1, in_=wo_f1)
            wo_b2 = wpool.tile([DK, D], BF, tag="wo_b2")
            nc.any.tensor_copy(out=wo_b2, in_=wo_f2)
            nc.vector.memset(V[:, :, 64 : 3 * 65 : 65], 1.0)

            bias_a = {}
            bias_b = {}
            if use_bias_qkv:
                for i, name in enumerate(("wq", "wk", "wv")):
                    ba = wpool.tile([P, 1], F32, tag=f"ba_{name}", name=f"ba_{name}")
                    nc.sync.dma_start(out=ba, in_=bqkv[i, 0:P].rearrange("p -> p 1"))
                    bb = wpool.tile([DK, 1], F32, tag=f"bb_{name}", name=f"bb_{name}")
                    nc.sync.dma_start(out=bb, in_=bqkv[i, P:HD].rearrange("p -> p 1"))
                    bias_a[name] = ba
                    bias_b[name] = bb

            def v_proj():
                # V natural: [128(s), 192] per s-tile = x @ Wv.  Emitted
                # between the K and Q projections: it has no DMA dependency
                # left by then, so it keeps the PE dense (HAM warm) while
                # the xq chunks stream in.
                wb = w_bf["wv"]
                for st in range(NS):
                    psV = mm_pool.tile([P, HD], F32, tag="mm", name="psV")
                    for d in range(ND):
                        nc.tensor.matmul(
                            psV, xtc[("wv", d)][:, st * P : (st + 1) * P], wb[:, d, :],
                            start=(d == 0), stop=(d == ND - 1),
                        )
                    for h in range(HG):
                        nc.any.tensor_copy(
                            out=V[:, st, h * 65 : h * 65 + 64],
                            in_=psV[:, h * DK : (h + 1) * DK],
                        )

            # K^T / Q^T: d-outer over s-block pairs (stationary W reused,
            # chunks consumed as their DMAs land)
            for name, dstA, dstB in (("wk", KTa, KTb), ("wq", QTa, QTb)):
                if name == "wq":
                    v_proj()
                wb = w_bf[name]
                for sbp in range(NB // 2):
                    ssl = slice(sbp * QH, (sbp + 1) * QH)
                    psA = qka_pool.tile([P, QH], F32, tag="qka", name="psA")
                    psB = qkb_pool.tile([DK, QH], F32, tag="qkb", name="psB")
                    for d in range(ND):
                        xt_d = xtc[(name, d)]
                        for half in range(2):
                            hsl = slice(half * 512, (half + 1) * 512)
                            xsl = slice(sbp * QH + half * 512, sbp * QH + (half + 1) * 512)
                            nc.tensor.matmul(
                                psA[:, hsl], wb[:, d, 0:P], xt_d[:, xsl],
                                start=(d == 0), stop=(d == ND - 1),
                            )
                        for half in range(2):
                            hsl = slice(half * 512, (half + 1) * 512)
                            xsl = slice(sbp * QH + half * 512, sbp * QH + (half + 1) * 512)
                            nc.tensor.matmul(
                                psB[:, hsl], wb[:, d, P:HD], xt_d[:, xsl],
                                start=(d == 0), stop=(d == ND - 1),
                            )
                    if use_bias_qkv:
                        nc.vector.tensor_scalar_add(dstA[:, ssl], psA, bias_a[name])
                        nc.vector.tensor_scalar_add(dstB[:, ssl], psB, bias_b[name])
                    else:
                        nc.any.tensor_copy(out=dstA[:, ssl], in_=psA)
                        nc.any.tensor_copy(out=dstB[:, ssl], in_=psB)

        # ============ phase 2+3: attention (+ interleaved Wo) ============
        with tc.tile_pool(name="s_ps", bufs=2, space="PSUM") as s_pool, \
             tc.tile_pool(name="ot_ps", bufs=2, space="PSUM") as ot_pool, \
             tc.tile_pool(name="pt", bufs=3) as pt_pool, \
             tc.tile_pool(name="nrm", bufs=2) as nrm_pool, \
             tc.tile_pool(name="y_sb", bufs=2) as ysb_pool:

            def scores(h, kt, qh):
                tp = (0, 0) if h == 0 else ((64, 0) if h == 1 else None)
                s_ps = s_pool.tile([P, QH], F32, tag="s", name="s_ps")
                for n in range(QH // 512):
                    q0 = qh * QH + n * 512
                    nc.tensor.matmul(
                        s_ps[:, n * 512 : (n + 1) * 512],
                        KT[h][:, kt * P : (kt + 1) * P],
                        QT[h][:, q0 : q0 + 512],
                        start=True, stop=True, tile_position=tp,
                    )
                pt = pt_pool.tile([P, QH], BF, tag="pt", name="pt")
                nc.scalar.activation(pt, s_ps, EXP, bias=0.0, scale=0.125)
                return pt

            def pv(h, kt, ot, pt):
                for n in range(QH // 512):
                    nc.tensor.matmul(
                        ot[:, n * 512 : (n + 1) * 512],
                        V[:, kt, h * 65 : (h + 1) * 65],
                        pt[:, n * 512 : (n + 1) * 512],
                        start=(kt == 0), stop=(kt == NS - 1),
                    )

            def normalize(h, qh, ot):
                osb = nrm_pool.tile([DK, QH], F32, tag="osb", name="osb")
                nc.vector.tensor_copy(out=osb, in_=ot[0:DK, :])
                den = nrm_pool.tile([1, QH], F32, tag="den", name="den")
                nc.vector.tensor_copy(out=den, in_=ot[64:65, :])
                recip = nrm_pool.tile([1, QH], F32, tag="recip", name="recip")
                nc.vector.reciprocal_approx_fast(recip, den)
                rbc = nrm_pool.tile([DK, QH], F32, tag="rbc", name="rbc")
                nc.gpsimd.partition_broadcast(rbc, recip)
                sl = slice(qh * QH, (qh + 1) * QH)
                dst = OC1[0:DK, sl] if h == 0 else (OC1[DK:P, sl] if h == 1 else OC2[:, sl])
                nc.vector.tensor_mul(dst, osb, rbc)

            y_r = y.rearrange("(n p) m -> n p m", p=P)

            def wo_tile(st):
                y_ps = ot_pool.tile([P, D], F32, tag="ot", name="y_ps")
                sl = slice(st * P, (st + 1) * P)
                for n0, nn in ((0, 512), (512, 256)):
                    nc.tensor.matmul(
                        y_ps[:, n0 : n0 + nn], OC1[:, sl], wo_b1[:, n0 : n0 + nn],
                        start=True, stop=False,
                    )
                    nc.tensor.matmul(
                        y_ps[:, n0 : n0 + nn], OC2[:, sl], wo_b2[:, n0 : n0 + nn],
                        start=False, stop=True,
                    )
                y_sb = ysb_pool.tile([P, D], F32, tag="ysb", name="y_sb")
                nc.vector.tensor_copy(out=y_sb, in_=y_ps)
                nc.sync.dma_start(out=y_r[st], in_=y_sb)

            for qh in range(S // QH):
                # paired heads 0,1: both scores before both PVs so the
                # in-order PE queue never stalls behind an exp wait
                ot01 = [
                    ot_pool.tile([65, QH], F32, tag="ot", name=f"ot{h}_{qh}")
                    for h in range(2)
                ]
                def scores_pair(kt):
                    sps = [s_pool.tile([P, QH], F32, tag="s", name=f"s_ps{h}") for h in range(2)]
                    for n in range(QH // 512):
                        q0 = qh * QH + n * 512
                        for h in range(2):
                            nc.tensor.matmul(
                                sps[h][:, n * 512 : (n + 1) * 512],
                                KT[h][:, kt * P : (kt + 1) * P],
                                QT[h][:, q0 : q0 + 512],
                                start=True, stop=True,
                                tile_position=(64 * h, 0),
                            )
                    out = []
                    for h in range(2):
                        pt = pt_pool.tile([P, QH], BF, tag="pt", name="pt")
                        nc.scalar.activation(pt, sps[h], EXP, bias=0.0, scale=0.125)
                        out.append(pt)
                    return out

                pts = scores_pair(0)
                for kt in range(NS):
                    nxt = scores_pair(kt + 1) if kt < NS - 1 else [None, None]
                    for h in range(2):
                        pv(h, kt, ot01[h], pts[h])
                    pts = nxt

                for h in range(2):
                    normalize(h, qh, ot01[h])
                # solo head 2, software-pipelined, with the previous
                # q-half's Wo tiles interleaved as PE filler
                ot2 = ot_pool.tile([65, QH], F32, tag="ot", name=f"ot2_{qh}")
                pt2 = scores(2, 0, qh)
                for kt in range(NS):
                    nxt = scores(2, kt + 1, qh) if kt < NS - 1 else None
                    pv(2, kt, ot2, pt2)
                    pt2 = nxt
                    if qh > 0 and kt % 2 == 1:
                        wo_tile((qh - 1) * (NS // 2) + kt // 2)
                normalize(2, qh, ot2)
            for st in range(NS // 2, NS):
                wo_tile(st)

    nc.compile()
    return nc


def kernel(query, key, value, Wq, bq, Wk, bk, Wv, bv, Wo, bo, **_ignored):
    from concourse.bass_utils import run_bass_kernel_spmd

    query = np.asarray(query, dtype=np.float32)
    key = np.asarray(key, dtype=np.float32)
    value = np.asarray(value, dtype=np.float32)
    Wq = np.asarray(Wq, dtype=np.float32)
    Wk = np.asarray(Wk, dtype=np.float32)
    Wv = np.asarray(Wv, dtype=np.float32)
    Wo = np.asarray(Wo, dtype=np.float32)
    bq = np.asarray(bq, dtype=np.float32)
    bk = np.asarray(bk, dtype=np.float32)
    bv = np.asarray(bv, dtype=np.float32)
    bo = np.asarray(bo, dtype=np.float32)

    use_bias_qkv = bool(np.any(bq) or np.any(bk) or np.any(bv))
    if "nc" not in _CACHE or _CACHE.get("bias") != use_bias_qkv:
        _CACHE["nc"] = _build_nc(use_bias_qkv)
        _CACHE["bias"] = use_bias_qkv
    nc = _CACHE["nc"]

    xT = {b: {} for b in range(B)}
    for b in range(B):
        xT[b]["q"] = np.ascontiguousarray(query[b].T)
        xT[b]["k"] = np.ascontiguousarray(key[b].T)
        xT[b]["v"] = np.ascontiguousarray(value[b].T)

    in_maps = []
    for c in range(8):
        b, g = divmod(c, 4)
        hs = slice(g * HD, (g + 1) * HD)
        in_maps.append({
            "xqT": xT[b]["q"],
            "xkT": xT[b]["k"],
            "xvT": xT[b]["v"],
            "wq": np.ascontiguousarray(Wq[:, hs]),
            "wk": np.ascontiguousarray(Wk[:, hs]),
            "wv": np.ascontiguousarray(Wv[:, hs]),
            "wo": np.ascontiguousarray(Wo[hs, :]),
            "bqkv": np.ascontiguousarray(
                np.stack([bq[hs], bk[hs], bv[hs]]).astype(np.float32)
            ),
        })

    res = run_bass_kernel_spmd(nc, in_maps, core_ids=list(range(8)), **_CACHE.get("run_kwargs", {}))
    _CACHE["last_result"] = res

    out = np.empty((B, S, D), dtype=np.float32)
    for b in range(B):
        acc = res.results[4 * b]["y"].astype(np.float32).copy()
        for g in range(1, 4):
            acc += res.results[4 * b + g]["y"]
        out[b] = acc + bo[None, :]
    return out



# revision 22
# speedup vs baseline: 1.1339x; 1.1339x over previous
"""Multi-head attention (B=2, S=2048, D=768, H=12) on 8 Trainium2 cores.

Sharding: core c -> batch b = c // 4, head-group g = c % 4 (3 heads of 12).
Host pre-transposes activations (x^T [768, 2048] fp32) and slices weight
shards; device computes Q/K/V projections, attention, and the head-group's
partial output through its Wo row shard; host sums 4 partials per batch
and adds bo.

Schedule (per core) - built around two facts measured on HW:
  * the Scalar engine's 96 exp tiles (12.6M elems) are an ~89us floor that
    must run as one gapless stream;
  * HBM-in (21.3MB fp32) is a ~59us stream, so inputs load in need-order
    on the sync HW-DGE queue: xk[k<1024], xq[q<1024], xk[k>=1024],
    xv, xq[q>=1024].
Projections run in float32r straight off the raw fp32 chunks (1 cycle/row
at free>=256, no cast DMA needed).  Attention per q-half: heads 0/1 are
row-tiled pairs - each kt's scores for both heads land packed in one
[128,1024] PSUM tile ([h0|h1] x 512q), one exp covers both; head 2 pairs
with itself across kt (K2/Q2 duplicated to partitions 64-127 by SBUF->SBUF
DMA).  PV chases the exp stream off the critical path; Wo for each half
and the Q-half-1 projection are interleaved into later windows where PSUM
banks free up.  Every copy/normalize is pinned to vector/gpsimd so ACT
runs exps only.
"""

import sys

for _p in ("/opt/trn_rl_repo",):
    if _p not in sys.path:
        sys.path.append(_p)

import numpy as np

B = 2
S = 2048
D = 768
H = 12
DK = 64
HG = 3            # heads per core
HD = HG * DK      # 192
P = 128
NS = S // P       # 16 k-tiles
ND = D // P       # 6 d-chunks
QH = 1024         # q half
VP = 256          # padded V-projection width (f32r needs free>=256)

_CACHE = {}


def _build_nc(use_bias_qkv, debug_dump=False):
    import concourse.bacc as bacc
    import concourse.tile as tile
    from concourse import mybir
    from contextlib import ExitStack

    BF = mybir.dt.bfloat16
    F32 = mybir.dt.float32
    FR = mybir.dt.float32r
    EXP = mybir.ActivationFunctionType.Exp

    nc = bacc.Bacc("TRN2", target_bir_lowering=False, debug=False)

    xqT = nc.dram_tensor("xqT", [D, S], FR, kind="ExternalInput").ap()
    xkT = nc.dram_tensor("xkT", [D, S], FR, kind="ExternalInput").ap()
    xvT = nc.dram_tensor("xvT", [D, S], FR, kind="ExternalInput").ap()
    wq = nc.dram_tensor("wq", [D, HD], FR, kind="ExternalInput").ap()
    wk = nc.dram_tensor("wk", [D, HD], FR, kind="ExternalInput").ap()
    wv = nc.dram_tensor("wv", [D, VP], FR, kind="ExternalInput").ap()  # padded
    wo = nc.dram_tensor("wo", [HD, D], F32, kind="ExternalInput").ap()
    bqkv = nc.dram_tensor("bqkv", [3, HD], F32, kind="ExternalInput").ap()
    y = nc.dram_tensor("y", [S, D], F32, kind="ExternalOutput").ap()
    y_r = y.rearrange("(n p) m -> n p m", p=P)
    dbg = {}
    if debug_dump:
        for nm, shp in (("dQTa", [P, S]), ("dKTa", [P, S]), ("dQTb", [P, S]),
                        ("dKTb", [P, S]), ("dV", [P, NS * 3 * 65]),
                        ("dOC1", [P, S]), ("dOC2", [DK, S]),
                        ("dPT0", [P, QH]), ("dPT2", [P, QH])):
            dbg[nm] = nc.dram_tensor(nm, shp, mybir.dt.bfloat16,
                                     kind="ExternalOutput").ap()
        for nm in ("dOT0", "dOT1", "dOT2"):
            dbg[nm] = nc.dram_tensor(nm, [65, 512], F32,
                                     kind="ExternalOutput").ap()

    with tile.TileContext(nc) as tc, ExitStack() as ctx:
        wpool = ctx.enter_context(tc.tile_pool(name="weights", bufs=1))
        apool = ctx.enter_context(tc.tile_pool(name="acts", bufs=1))

        # persistent activations
        QTa = apool.tile([P, S], BF, tag="qta")    # heads 0,1 stacked (Q^T)
        KTa = apool.tile([P, S], BF, tag="kta")
        QTb = apool.tile([P, S], BF, tag="qtb")    # head 2 rows 0:64, dup 64:128
        KTb = apool.tile([P, S], BF, tag="ktb")
        V = apool.tile([P, NS, 3 * 65], BF, tag="v")
        OC1 = apool.tile([P, S], BF, tag="oc1")    # normalized O^T heads 0,1
        OC2 = apool.tile([DK, S], BF, tag="oc2")   # head 2

        # ---------------- DMA: weights on scalar HW queue ----------------
        wkb = wpool.tile([P, ND, HD], FR, tag="wk")
        nc.scalar.dma_start(out=wkb, in_=wk.rearrange("(nd p) h -> p nd h", p=P))
        wqb = wpool.tile([P, ND, HD], FR, tag="wq")
        nc.scalar.dma_start(out=wqb, in_=wq.rearrange("(nd p) h -> p nd h", p=P))
        wvb = wpool.tile([P, ND, VP], FR, tag="wv")
        nc.scalar.dma_start(out=wvb, in_=wv.rearrange("(nd p) h -> p nd h", p=P))
        wo_f1 = wpool.tile([P, D], F32, tag="wo_f1")
        nc.scalar.dma_start(out=wo_f1, in_=wo[0:P, :])
        wo_f2 = wpool.tile([DK, D], F32, tag="wo_f2")
        nc.scalar.dma_start(out=wo_f2, in_=wo[P:HD, :])
        wo_b1 = wpool.tile([P, D], BF, tag="wo_b1")
        nc.vector.tensor_copy(out=wo_b1, in_=wo_f1)
        wo_b2 = wpool.tile([DK, D], BF, tag="wo_b2")
        nc.vector.tensor_copy(out=wo_b2, in_=wo_f2)
        nc.vector.memset(V[:, :, 64 : 3 * 65 : 65], 1.0)

        bias_a = {}
        bias_b = {}
        if use_bias_qkv:
            for i, name in enumerate(("wq", "wk", "wv")):
                ba = wpool.tile([P, 1], F32, tag=f"ba_{name}", name=f"ba_{name}")
                nc.scalar.dma_start(out=ba, in_=bqkv[i, 0:P].rearrange("p -> p 1"))
                bb = wpool.tile([DK, 1], F32, tag=f"bb_{name}", name=f"bb_{name}")
                nc.scalar.dma_start(out=bb, in_=bqkv[i, P:HD].rearrange("p -> p 1"))
                bias_a[name] = ba
                bias_b[name] = bb
            # V bias as a row, broadcast across partitions
            bvrow = wpool.tile([1, HD], F32, tag="bvrow")
            nc.scalar.dma_start(out=bvrow, in_=bqkv[2, :].rearrange("h -> 1 h"))
            bvbc = wpool.tile([P, HD], F32, tag="bvbc")
            nc.gpsimd.partition_broadcast(bvbc, bvrow)

        # -------- DMA: x in need-order on sync HW queue (raw fp32) --------
        # half-chunks [128, 1024]; order: xk(s<1024), xq(q<1024), xk(s>=1024),
        # xv(s<1024), xv(s>=1024), xq(q>=1024)
        xpool = ctx.enter_context(tc.tile_pool(name="xt", bufs=3))
        xh = {}

        def load_half(name, xT, h):
            for dc in range(ND):
                t = xpool.tile([P, QH], FR, tag=f"x{dc}", name=f"x_{name}{dc}_{h}")
                nc.sync.dma_start(
                    out=t, in_=xT[dc * P : (dc + 1) * P, h * QH : (h + 1) * QH]
                )
                xh[(name, dc, h)] = t

        load_half("k", xkT, 0)
        load_half("q", xqT, 0)
        load_half("k", xkT, 1)
        load_half("v", xvT, 0)
        load_half("v", xvT, 1)
        load_half("q", xqT, 1)

        # ---------------- projections (float32r) ----------------
        def qk_proj_big(name, wb, dstA, dstB, h, pool_a, pool_b):
            """One s-half of Q^T/K^T via [128,1024]+[64,1024] psum tiles."""
            psA = pool_a.tile([P, QH], F32, tag="pjA", name=f"pjA_{name}{h}")
            psB = pool_b.tile([DK, QH], F32, tag="pjB", name=f"pjB_{name}{h}")
            for sec in range(2):
                ssl = slice(sec * 512, (sec + 1) * 512)
                for d in range(ND):
                    xt = xh[(name, d, h)]
                    nc.tensor.matmul(
                        psA[:, ssl], wb[:, d, 0:P], xt[:, ssl],
                        start=(d == 0), stop=(d == ND - 1),
                    )
                gsl = slice(h * QH + sec * 512, h * QH + (sec + 1) * 512)
                if use_bias_qkv:
                    nc.vector.tensor_scalar_add(dstA[:, gsl], psA[:, ssl], bias_a[name])
                else:
                    nc.vector.tensor_copy(out=dstA[:, gsl], in_=psA[:, ssl])
            for sec in range(2):
                ssl = slice(sec * 512, (sec + 1) * 512)
                for d in range(ND):
                    xt = xh[(name, d, h)]
                    nc.tensor.matmul(
                        psB[:, ssl], wb[:, d, P:HD], xt[:, ssl],
                        start=(d == 0), stop=(d == ND - 1),
                    )
                gsl = slice(h * QH + sec * 512, h * QH + (sec + 1) * 512)
                if use_bias_qkv:
                    nc.vector.tensor_scalar_add(
                        dstB[0:DK, gsl], psB[:, ssl], bias_b[name]
                    )
                else:
                    nc.vector.tensor_copy(out=dstB[0:DK, gsl], in_=psB[:, ssl])

        def qk_proj_small(name, wb, dstA, dstB, h, pool_s):
            """One s-half via single-bank [*,512] psum tiles (low PSUM)."""
            for sec in range(2):
                gsl = slice(h * QH + sec * 512, h * QH + (sec + 1) * 512)
                ssl = slice(sec * 512, (sec + 1) * 512)
                psA = pool_s.tile([P, 512], F32, tag="pjS", name=f"pjS_{name}{h}{sec}")
                for d in range(ND):
                    nc.tensor.matmul(
                        psA, wb[:, d, 0:P], xh[(name, d, h)][:, ssl],
                        start=(d == 0), stop=(d == ND - 1),
                    )
                if use_bias_qkv:
                    nc.vector.tensor_scalar_add(dstA[:, gsl], psA, bias_a[name])
                else:
                    nc.vector.tensor_copy(out=dstA[:, gsl], in_=psA)
                psB = pool_s.tile([DK, 512], F32, tag="pjS", name=f"pjSb_{name}{h}{sec}")
                for d in range(ND):
                    nc.tensor.matmul(
                        psB, wb[:, d, P:HD], xh[(name, d, h)][:, ssl],
                        start=(d == 0), stop=(d == ND - 1),
                    )
                if use_bias_qkv:
                    nc.vector.tensor_scalar_add(dstB[0:DK, gsl], psB, bias_b[name])
                else:
                    nc.vector.tensor_copy(out=dstB[0:DK, gsl], in_=psB)

        def v_proj_half(h, pool_v):
            """V natural [s, 192(+pad)] for s-tiles of one half; 1-bank psum."""
            for st in range(h * 8, (h + 1) * 8):
                psV = pool_v.tile([P, VP], F32, tag="pv", name=f"pv{st}")
                for d in range(ND):
                    nc.tensor.matmul(
                        psV,
                        xh[("v", d, h)][:, (st - h * 8) * P : (st - h * 8 + 1) * P],
                        wvb[:, d, :],
                        start=(d == 0), stop=(d == ND - 1),
                    )
                # one strided copy: 3 heads x 64 cols into 65-strided V
                nc.vector.tensor_copy(
                    out=V[:, st, :].rearrange("p (g c) -> p g c", c=65)[:, :, 0:64],
                    in_=psV[:, 0:HD].rearrange("p (g c) -> p g c", c=64),
                )
                if use_bias_qkv:
                    for g in range(HG):
                        nc.vector.tensor_add(
                            V[:, st, g * 65 : g * 65 + 64],
                            V[:, st, g * 65 : g * 65 + 64],
                            bvbc[:, g * DK : (g + 1) * DK],
                        )

        # K2/Q2 duplication to partitions 64..127 (SBUF->SBUF DMA, per half)
        def dup_half(h):
            sl = slice(h * QH, (h + 1) * QH)
            nc.scalar.dma_start(out=KTb[DK:P, sl], in_=KTb[0:DK, sl])
            nc.scalar.dma_start(out=QTb[DK:P, sl], in_=QTb[0:DK, sl])

        # ---------------- attention building blocks ----------------
        def normalize(ot, dst_sl, h2):
            """ot [65, 512] psum -> OC (bf16), dividing by ot[64] (denom)."""
            npool = nrm_pool
            osb = npool.tile([DK, 512], F32, tag="osb", name="osb")
            nc.vector.tensor_copy(out=osb, in_=ot[0:DK, :])
            den = npool.tile([1, 512], F32, tag="den", name="den")
            nc.vector.tensor_copy(out=den, in_=ot[64:65, :])
            recip = npool.tile([1, 512], F32, tag="recip", name="recip")
            nc.vector.reciprocal_approx_fast(recip, den)
            rbc = npool.tile([DK, 512], F32, tag="rbc", name="rbc")
            nc.gpsimd.partition_broadcast(rbc, recip)
            if h2:
                nc.vector.tensor_mul(OC2[:, dst_sl], osb, rbc)
            else:
                nc.vector.tensor_mul(OC1[dst_sl[0], dst_sl[1]], osb, rbc)

        def wo_tile(st):
            y_ps = pools["wo"].tile([P, D], F32, tag="yps", name=f"yps{st}")
            sl = slice(st * P, (st + 1) * P)
            for n0, nn in ((0, 512), (512, 256)):
                nc.tensor.matmul(
                    y_ps[:, n0 : n0 + nn], OC1[:, sl], wo_b1[:, n0 : n0 + nn],
                    start=True, stop=False,
                )
                nc.tensor.matmul(
                    y_ps[:, n0 : n0 + nn], OC2[:, sl], wo_b2[:, n0 : n0 + nn],
                    start=False, stop=True,
                )
            y_sb = ysb_pool.tile([P, D], F32, tag="ysb", name=f"ysb{st}")
            nc.vector.tensor_copy(out=y_sb, in_=y_ps)
            nc.sync.dma_start(out=y_r[st], in_=y_sb)

        def pair_subsweep(qh, sec, filler):
            """Heads 0,1 over all kt for one 512-q section.  One packed
            [128,1024] score tile per kt ([h0|h1]); exp covers both."""
            q0 = qh * QH + sec * 512
            qsl = slice(q0, q0 + 512)
            ot0 = ot_pool.tile([65, 512], F32, tag="ot", name=f"ot0_{qh}{sec}")
            ot1 = ot_pool.tile([65, 512], F32, tag="ot", name=f"ot1_{qh}{sec}")
            pts = []
            for kt in range(NS):
                ksl = slice(kt * P, (kt + 1) * P)
                sp = s_pool.tile([P, QH], F32, tag="s", name=f"sp{qh}{sec}{kt}")
                nc.tensor.matmul(
                    sp[:, 0:512], KTa[0:DK, ksl], QTa[0:DK, qsl],
                    start=True, stop=True, tile_position=(0, 0),
                )
                nc.tensor.matmul(
                    sp[:, 512:QH], KTa[DK:P, ksl], QTa[DK:P, qsl],
                    start=True, stop=True, tile_position=(64, 0),
                )
                pt = pt_pool.tile([P, QH], BF, tag="pt", name="pt")
                nc.scalar.activation(pt, sp, EXP, bias=0.0, scale=0.125)
                if debug_dump and qh == 0 and sec == 0 and kt == 0:
                    nc.sync.dma_start(out=dbg["dPT0"], in_=pt)
                pts.append(pt)
                # PV chases one kt behind so exp(kt+1) never waits on PE
                if kt >= 1:
                    pv_pair(kt - 1, pts[kt - 1], ot0, ot1)
                    pts[kt - 1] = None
                if filler and kt % 4 == 3:
                    filler(kt // 4)
            pv_pair(NS - 1, pts[NS - 1], ot0, ot1)
            if debug_dump and qh == 0 and sec == 0:
                for nm, ot in (("dOT0", ot0), ("dOT1", ot1)):
                    osb = nrm_pool.tile([65, 512], F32, tag="dbg", name=nm)
                    nc.vector.tensor_copy(out=osb, in_=ot)
                    nc.sync.dma_start(out=dbg[nm], in_=osb)
            normalize(ot0, (slice(0, DK), qsl), False)
            normalize(ot1, (slice(DK, P), qsl), False)

        def pv_pair(kt, pt, ot0, ot1):
            nc.tensor.matmul(
                ot0, V[:, kt, 0:65], pt[:, 0:512],
                start=(kt == 0), stop=(kt == NS - 1),
            )
            nc.tensor.matmul(
                ot1, V[:, kt, 65:130], pt[:, 512:QH],
                start=(kt == 0), stop=(kt == NS - 1),
            )

        def h2_subsweep(qh, sec, filler):
            """Head 2 paired with itself across kt: packed tile is
            [kt_even | kt_odd] for the same 512-q section."""
            q0 = qh * QH + sec * 512
            qsl = slice(q0, q0 + 512)
            ot2 = ot_pool.tile([65, 512], F32, tag="ot", name=f"ot2_{qh}{sec}")
            pts = []
            for kp in range(NS // 2):
                ke, ko = 2 * kp, 2 * kp + 1
                sp = s_pool.tile([P, QH], F32, tag="s", name=f"s2{qh}{sec}{kp}")
                nc.tensor.matmul(
                    sp[:, 0:512], KTb[0:DK, ke * P : (ke + 1) * P], QTb[0:DK, qsl],
                    start=True, stop=True, tile_position=(0, 0),
                )
                nc.tensor.matmul(
                    sp[:, 512:QH], KTb[DK:P, ko * P : (ko + 1) * P], QTb[DK:P, qsl],
                    start=True, stop=True, tile_position=(64, 0),
                )
                pt = pt_pool.tile([P, QH], BF, tag="pt", name="pt")
                nc.scalar.activation(pt, sp, EXP, bias=0.0, scale=0.125)
                if debug_dump and qh == 0 and sec == 0 and kp == 0:
                    nc.sync.dma_start(out=dbg["dPT2"], in_=pt)
                pts.append(pt)
                if kp >= 1:
                    pv2(kp - 1, pts[kp - 1], ot2)
                    pts[kp - 1] = None
                if filler:
                    filler(kp)
            pv2(NS // 2 - 1, pts[NS // 2 - 1], ot2)
            if debug_dump and qh == 0 and sec == 0:
                osb = nrm_pool.tile([65, 512], F32, tag="dbg", name="dOT2")
                nc.vector.tensor_copy(out=osb, in_=ot2)
                nc.sync.dma_start(out=dbg["dOT2"], in_=osb)
            normalize(ot2, qsl, True)

        def pv2(kp, pt, ot2):
            ke, ko = 2 * kp, 2 * kp + 1
            nc.tensor.matmul(
                ot2, V[:, ke, 130:195], pt[:, 0:512],
                start=(kp == 0), stop=False,
            )
            nc.tensor.matmul(
                ot2, V[:, ko, 130:195], pt[:, 512:QH],
                start=False, stop=(kp == NS // 2 - 1),
            )

        # ---------------- phase plumbing ----------------
        # PSUM budget (8 banks): scores 2x[128,1024]=4, ot 2x[65,512]=2,
        # plus 2 for either the small projection pool (q-half 0) or the
        # Wo pool (q-half 1 + tail) -- time-multiplexed via with-scopes.
        pt_pool = ctx.enter_context(tc.tile_pool(name="pt", bufs=4))
        nrm_pool = ctx.enter_context(tc.tile_pool(name="nrm", bufs=2))
        ysb_pool = ctx.enter_context(tc.tile_pool(name="ysb", bufs=2))
        pools = {}

        # K (k<1024) and Q (q<1024) projections via big tiles (PSUM free)
        with tc.tile_pool(name="pj_a", bufs=1, space="PSUM") as pool_a, \
             tc.tile_pool(name="pj_b", bufs=1, space="PSUM") as pool_b:
            qk_proj_big("k", wkb, KTa, KTb, 0, pool_a, pool_b)
            qk_proj_big("q", wqb, QTa, QTb, 0, pool_a, pool_b)

        dup_half(0)

        s_pool = ctx.enter_context(tc.tile_pool(name="s_ps", bufs=2, space="PSUM"))
        ot_pool = ctx.enter_context(tc.tile_pool(name="ot_ps", bufs=2, space="PSUM"))

        # Remaining projections are EMITTED here in dataflow order (reads
        # must follow writes in program order for dependency tracking);
        # the scheduler still overlaps them with the attention sweeps by
        # DMA readiness, using the small 2-bank pj pool.
        def dup_half_k(h):
            sl = slice(h * QH, (h + 1) * QH)
            nc.scalar.dma_start(out=KTb[DK:P, sl], in_=KTb[0:DK, sl])

        def dup_half_q(h):
            sl = slice(h * QH, (h + 1) * QH)
            nc.scalar.dma_start(out=QTb[DK:P, sl], in_=QTb[0:DK, sl])

        with tc.tile_pool(name="pj_s", bufs=1, space="PSUM") as pjp:
            pools["pj"] = pjp
            qk_proj_small("k", wkb, KTa, KTb, 1, pjp)
            dup_half_k(1)
            v_proj_half(0, pjp)
            v_proj_half(1, pjp)
            qk_proj_small("q", wqb, QTa, QTb, 1, pjp)
            dup_half_q(1)
            pair_subsweep(0, 0, None)
            pair_subsweep(0, 1, None)
            h2_subsweep(0, 0, None)
            h2_subsweep(0, 1, None)

        # ---- q-half 1: pair subsweeps with Wo(half0) interleaved ----
        with tc.tile_pool(name="wo_ps", bufs=1, space="PSUM") as wop:
            pools["wo"] = wop
            pair_subsweep(1, 0, lambda i: wo_tile(i) if i < 4 else None)
            pair_subsweep(1, 1, lambda i: wo_tile(4 + i) if i < 4 else None)

            # h2 of half 1; remaining wo tiles interleave after normalizes
            # complete (s-tiles 8..11 during sec B, 12..15 at tail)
            h2_subsweep(1, 0, None)

            def fill_h2_1B(i):
                if i < 4:
                    wo_tile(8 + i)

            h2_subsweep(1, 1, fill_h2_1B)
            for st in range(12, NS):
                wo_tile(st)

        if debug_dump:
            nc.sync.dma_start(out=dbg["dQTa"], in_=QTa)
            nc.sync.dma_start(out=dbg["dKTa"], in_=KTa)
            nc.sync.dma_start(out=dbg["dQTb"], in_=QTb)
            nc.sync.dma_start(out=dbg["dKTb"], in_=KTb)
            nc.sync.dma_start(out=dbg["dV"], in_=V.rearrange("p a b -> p (a b)"))
            nc.sync.dma_start(out=dbg["dOC1"], in_=OC1)
            nc.sync.dma_start(out=dbg["dOC2"], in_=OC2)

    nc.compile()
    return nc


def kernel(query, key, value, Wq, bq, Wk, bk, Wv, bv, Wo, bo, **_ignored):
    from concourse.bass_utils import run_bass_kernel_spmd

    query = np.asarray(query, dtype=np.float32)
    key = np.asarray(key, dtype=np.float32)
    value = np.asarray(value, dtype=np.float32)
    Wq = np.asarray(Wq, dtype=np.float32)
    Wk = np.asarray(Wk, dtype=np.float32)
    Wv = np.asarray(Wv, dtype=np.float32)
    Wo = np.asarray(Wo, dtype=np.float32)
    bq = np.asarray(bq, dtype=np.float32)
    bk = np.asarray(bk, dtype=np.float32)
    bv = np.asarray(bv, dtype=np.float32)
    bo = np.asarray(bo, dtype=np.float32)

    use_bias_qkv = bool(np.any(bq) or np.any(bk) or np.any(bv))
    if "nc" not in _CACHE or _CACHE.get("bias") != use_bias_qkv:
        _CACHE["nc"] = _build_nc(use_bias_qkv)
        _CACHE["bias"] = use_bias_qkv
    nc = _CACHE["nc"]

    xT = {b: {} for b in range(B)}
    for b in range(B):
        xT[b]["q"] = np.ascontiguousarray(query[b].T)
        xT[b]["k"] = np.ascontiguousarray(key[b].T)
        xT[b]["v"] = np.ascontiguousarray(value[b].T)

    in_maps = []
    for c in range(8):
        b, g = divmod(c, 4)
        hs = slice(g * HD, (g + 1) * HD)
        wv_p = np.zeros((D, VP), dtype=np.float32)
        wv_p[:, 0:HD] = Wv[:, hs]
        in_maps.append({
            "xqT": xT[b]["q"],
            "xkT": xT[b]["k"],
            "xvT": xT[b]["v"],
            "wq": np.ascontiguousarray(Wq[:, hs]),
            "wk": np.ascontiguousarray(Wk[:, hs]),
            "wv": wv_p,
            "wo": np.ascontiguousarray(Wo[hs, :]),
            "bqkv": np.ascontiguousarray(
                np.stack([bq[hs], bk[hs], bv[hs]]).astype(np.float32)
            ),
        })

    res = run_bass_kernel_spmd(nc, in_maps, core_ids=list(range(8)), **_CACHE.get("run_kwargs", {}))
    _CACHE["last_result"] = res

    out = np.empty((B, S, D), dtype=np.float32)
    for b in range(B):
        acc = res.results[4 * b]["y"].astype(np.float32).copy()
        for g in range(1, 4):
            acc += res.results[4 * b + g]["y"]
        out[b] = acc + bo[None, :]
    return out


# revision 23
# speedup vs baseline: 1.1371x; 1.0028x over previous
"""Multi-head attention (B=2, S=2048, D=768, H=12) on 8 Trainium2 cores.

Sharding: core c -> batch b = c // 4, head-group g = c % 4 (3 heads of 12).
Host pre-transposes activations (x^T [768, 2048] fp32) and slices weight
shards; device computes Q/K/V projections, attention, and the head-group's
partial output through its Wo row shard; host sums 4 partials per batch
and adds bo.

Schedule (per core) - built around two facts measured on HW:
  * the Scalar engine's 96 exp tiles (12.6M elems) are an ~89us floor that
    must run as one gapless stream;
  * HBM-in (21.3MB fp32) is a ~59us stream, so inputs load in need-order
    on the sync HW-DGE queue: xk[k<1024], xq[q<1024], xk[k>=1024],
    xv, xq[q>=1024].
Projections run in float32r straight off the raw fp32 chunks (1 cycle/row
at free>=256, no cast DMA needed).  Attention per q-half: heads 0/1 are
row-tiled pairs - each kt's scores for both heads land packed in one
[128,1024] PSUM tile ([h0|h1] x 512q), one exp covers both; head 2 pairs
with itself across kt (K2/Q2 duplicated to partitions 64-127 by SBUF->SBUF
DMA).  PV chases the exp stream off the critical path; Wo for each half
and the Q-half-1 projection are interleaved into later windows where PSUM
banks free up.  Every copy/normalize is pinned to vector/gpsimd so ACT
runs exps only.
"""

import sys

for _p in ("/opt/trn_rl_repo",):
    if _p not in sys.path:
        sys.path.append(_p)

import numpy as np

B = 2
S = 2048
D = 768
H = 12
DK = 64
HG = 3            # heads per core
HD = HG * DK      # 192
P = 128
NS = S // P       # 16 k-tiles
ND = D // P       # 6 d-chunks
QH = 1024         # q half
VP = 256          # padded V-projection width (f32r needs free>=256)

_CACHE = {}


def _build_nc(use_bias_qkv, debug_dump=False):
    import concourse.bacc as bacc
    import concourse.tile as tile
    from concourse import mybir
    from contextlib import ExitStack

    BF = mybir.dt.bfloat16
    F32 = mybir.dt.float32
    FR = mybir.dt.float32r
    EXP = mybir.ActivationFunctionType.Exp

    nc = bacc.Bacc("TRN2", target_bir_lowering=False, debug=False)

    xqT = nc.dram_tensor("xqT", [D, S], FR, kind="ExternalInput").ap()
    xkT = nc.dram_tensor("xkT", [D, S], FR, kind="ExternalInput").ap()
    xvT = nc.dram_tensor("xvT", [D, S], FR, kind="ExternalInput").ap()
    wq = nc.dram_tensor("wq", [D, HD], FR, kind="ExternalInput").ap()
    wk = nc.dram_tensor("wk", [D, HD], FR, kind="ExternalInput").ap()
    wv = nc.dram_tensor("wv", [D, VP], FR, kind="ExternalInput").ap()  # padded
    wo = nc.dram_tensor("wo", [HD, D], F32, kind="ExternalInput").ap()
    bqkv = nc.dram_tensor("bqkv", [3, HD], F32, kind="ExternalInput").ap()
    y = nc.dram_tensor("y", [S, D], F32, kind="ExternalOutput").ap()
    y_r = y.rearrange("(n p) m -> n p m", p=P)
    dbg = {}
    if debug_dump:
        for nm, shp in (("dQTa", [P, S]), ("dKTa", [P, S]), ("dQTb", [P, S]),
                        ("dKTb", [P, S]), ("dV", [P, NS * 3 * 65]),
                        ("dOC1", [P, S]), ("dOC2", [DK, S]),
                        ("dPT0", [P, QH]), ("dPT2", [P, QH])):
            dbg[nm] = nc.dram_tensor(nm, shp, mybir.dt.bfloat16,
                                     kind="ExternalOutput").ap()
        for nm in ("dOT0", "dOT1", "dOT2"):
            dbg[nm] = nc.dram_tensor(nm, [65, 512], F32,
                                     kind="ExternalOutput").ap()

    with tile.TileContext(nc) as tc, ExitStack() as ctx:
        wpool = ctx.enter_context(tc.tile_pool(name="weights", bufs=1))
        apool = ctx.enter_context(tc.tile_pool(name="acts", bufs=1))

        # persistent activations
        QTa = apool.tile([P, S], BF, tag="qta")    # heads 0,1 stacked (Q^T)
        KTa = apool.tile([P, S], BF, tag="kta")
        QTb = apool.tile([P, S], BF, tag="qtb")    # head 2 rows 0:64, dup 64:128
        KTb = apool.tile([P, S], BF, tag="ktb")
        V = apool.tile([P, NS, 3 * 65], BF, tag="v")
        OC1 = apool.tile([P, S], BF, tag="oc1")    # normalized O^T heads 0,1
        OC2 = apool.tile([DK, S], BF, tag="oc2")   # head 2

        # ---------------- DMA: weights on scalar HW queue ----------------
        wkb = wpool.tile([P, ND, HD], FR, tag="wk")
        nc.scalar.dma_start(out=wkb, in_=wk.rearrange("(nd p) h -> p nd h", p=P))
        wqb = wpool.tile([P, ND, HD], FR, tag="wq")
        nc.scalar.dma_start(out=wqb, in_=wq.rearrange("(nd p) h -> p nd h", p=P))
        wvb = wpool.tile([P, ND, VP], FR, tag="wv")
        nc.scalar.dma_start(out=wvb, in_=wv.rearrange("(nd p) h -> p nd h", p=P))
        wo_f1 = wpool.tile([P, D], F32, tag="wo_f1")
        nc.scalar.dma_start(out=wo_f1, in_=wo[0:P, :])
        wo_f2 = wpool.tile([DK, D], F32, tag="wo_f2")
        nc.scalar.dma_start(out=wo_f2, in_=wo[P:HD, :])
        wo_b1 = wpool.tile([P, D], BF, tag="wo_b1")
        nc.vector.tensor_copy(out=wo_b1, in_=wo_f1)
        wo_b2 = wpool.tile([DK, D], BF, tag="wo_b2")
        nc.vector.tensor_copy(out=wo_b2, in_=wo_f2)
        nc.vector.memset(V[:, :, 64 : 3 * 65 : 65], 1.0)

        bias_a = {}
        bias_b = {}
        if use_bias_qkv:
            for i, name in enumerate(("wq", "wk", "wv")):
                ba = wpool.tile([P, 1], F32, tag=f"ba_{name}", name=f"ba_{name}")
                nc.scalar.dma_start(out=ba, in_=bqkv[i, 0:P].rearrange("p -> p 1"))
                bb = wpool.tile([DK, 1], F32, tag=f"bb_{name}", name=f"bb_{name}")
                nc.scalar.dma_start(out=bb, in_=bqkv[i, P:HD].rearrange("p -> p 1"))
                bias_a[name] = ba
                bias_b[name] = bb
            # V bias as a row, broadcast across partitions
            bvrow = wpool.tile([1, HD], F32, tag="bvrow")
            nc.scalar.dma_start(out=bvrow, in_=bqkv[2, :].rearrange("h -> 1 h"))
            bvbc = wpool.tile([P, HD], F32, tag="bvbc")
            nc.gpsimd.partition_broadcast(bvbc, bvrow)

        # -------- DMA: x in need-order on sync HW queue (raw fp32) --------
        # half-chunks [128, 1024]; order: xk(s<1024), xq(q<1024), xk(s>=1024),
        # xv(s<1024), xv(s>=1024), xq(q>=1024)
        xpool = ctx.enter_context(tc.tile_pool(name="xt", bufs=4))
        xh = {}

        def load_half(name, xT, h):
            for dc in range(ND):
                t = xpool.tile([P, QH], FR, tag=f"x{dc}", name=f"x_{name}{dc}_{h}")
                nc.sync.dma_start(
                    out=t, in_=xT[dc * P : (dc + 1) * P, h * QH : (h + 1) * QH]
                )
                xh[(name, dc, h)] = t

        load_half("k", xkT, 0)
        load_half("q", xqT, 0)
        load_half("k", xkT, 1)
        load_half("v", xvT, 0)
        load_half("v", xvT, 1)
        load_half("q", xqT, 1)

        # ---------------- projections (float32r) ----------------
        def qk_proj_big(name, wb, dstA, dstB, h, pool_a, pool_b):
            """One s-half of Q^T/K^T via [128,1024]+[64,1024] psum tiles."""
            psA = pool_a.tile([P, QH], F32, tag="pjA", name=f"pjA_{name}{h}")
            psB = pool_b.tile([DK, QH], F32, tag="pjB", name=f"pjB_{name}{h}")
            for sec in range(2):
                ssl = slice(sec * 512, (sec + 1) * 512)
                for d in range(ND):
                    xt = xh[(name, d, h)]
                    nc.tensor.matmul(
                        psA[:, ssl], wb[:, d, 0:P], xt[:, ssl],
                        start=(d == 0), stop=(d == ND - 1),
                    )
                gsl = slice(h * QH + sec * 512, h * QH + (sec + 1) * 512)
                if use_bias_qkv:
                    nc.vector.tensor_scalar_add(dstA[:, gsl], psA[:, ssl], bias_a[name])
                else:
                    nc.vector.tensor_copy(out=dstA[:, gsl], in_=psA[:, ssl])
            for sec in range(2):
                ssl = slice(sec * 512, (sec + 1) * 512)
                for d in range(ND):
                    xt = xh[(name, d, h)]
                    nc.tensor.matmul(
                        psB[:, ssl], wb[:, d, P:HD], xt[:, ssl],
                        start=(d == 0), stop=(d == ND - 1),
                    )
                gsl = slice(h * QH + sec * 512, h * QH + (sec + 1) * 512)
                if use_bias_qkv:
                    nc.vector.tensor_scalar_add(
                        dstB[0:DK, gsl], psB[:, ssl], bias_b[name]
                    )
                else:
                    nc.vector.tensor_copy(out=dstB[0:DK, gsl], in_=psB[:, ssl])

        def qk_proj_small(name, wb, dstA, dstB, h, pool_s):
            """One s-half via single-bank [*,512] psum tiles (low PSUM)."""
            for sec in range(2):
                gsl = slice(h * QH + sec * 512, h * QH + (sec + 1) * 512)
                ssl = slice(sec * 512, (sec + 1) * 512)
                psA = pool_s.tile([P, 512], F32, tag="pjS", name=f"pjS_{name}{h}{sec}")
                for d in range(ND):
                    nc.tensor.matmul(
                        psA, wb[:, d, 0:P], xh[(name, d, h)][:, ssl],
                        start=(d == 0), stop=(d == ND - 1),
                    )
                if use_bias_qkv:
                    nc.vector.tensor_scalar_add(dstA[:, gsl], psA, bias_a[name])
                else:
                    nc.vector.tensor_copy(out=dstA[:, gsl], in_=psA)
                psB = pool_s.tile([DK, 512], F32, tag="pjS", name=f"pjSb_{name}{h}{sec}")
                for d in range(ND):
                    nc.tensor.matmul(
                        psB, wb[:, d, P:HD], xh[(name, d, h)][:, ssl],
                        start=(d == 0), stop=(d == ND - 1),
                    )
                if use_bias_qkv:
                    nc.vector.tensor_scalar_add(dstB[0:DK, gsl], psB, bias_b[name])
                else:
                    nc.vector.tensor_copy(out=dstB[0:DK, gsl], in_=psB)

        def v_proj_half(h, pool_v):
            """V natural [s, 192(+pad)] for s-tiles of one half; 1-bank psum."""
            for st in range(h * 8, (h + 1) * 8):
                psV = pool_v.tile([P, VP], F32, tag="pv", name=f"pv{st}")
                for d in range(ND):
                    nc.tensor.matmul(
                        psV,
                        xh[("v", d, h)][:, (st - h * 8) * P : (st - h * 8 + 1) * P],
                        wvb[:, d, :],
                        start=(d == 0), stop=(d == ND - 1),
                    )
                # one strided copy: 3 heads x 64 cols into 65-strided V
                nc.vector.tensor_copy(
                    out=V[:, st, :].rearrange("p (g c) -> p g c", c=65)[:, :, 0:64],
                    in_=psV[:, 0:HD].rearrange("p (g c) -> p g c", c=64),
                )
                if use_bias_qkv:
                    for g in range(HG):
                        nc.vector.tensor_add(
                            V[:, st, g * 65 : g * 65 + 64],
                            V[:, st, g * 65 : g * 65 + 64],
                            bvbc[:, g * DK : (g + 1) * DK],
                        )

        # K2/Q2 duplication to partitions 64..127 (SBUF->SBUF DMA, per half)
        def dup_half(h):
            sl = slice(h * QH, (h + 1) * QH)
            nc.scalar.dma_start(out=KTb[DK:P, sl], in_=KTb[0:DK, sl])
            nc.scalar.dma_start(out=QTb[DK:P, sl], in_=QTb[0:DK, sl])

        # ---------------- attention building blocks ----------------
        def normalize(ot, dst_sl, h2):
            """ot [65, 512] psum -> OC (bf16), dividing by ot[64] (denom)."""
            npool = nrm_pool
            osb = npool.tile([DK, 512], F32, tag="osb", name="osb")
            nc.vector.tensor_copy(out=osb, in_=ot[0:DK, :])
            den = npool.tile([1, 512], F32, tag="den", name="den")
            nc.vector.tensor_copy(out=den, in_=ot[64:65, :])
            recip = npool.tile([1, 512], F32, tag="recip", name="recip")
            nc.vector.reciprocal_approx_fast(recip, den)
            rbc = npool.tile([DK, 512], F32, tag="rbc", name="rbc")
            nc.gpsimd.partition_broadcast(rbc, recip)
            if h2:
                nc.vector.tensor_mul(OC2[:, dst_sl], osb, rbc)
            else:
                nc.vector.tensor_mul(OC1[dst_sl[0], dst_sl[1]], osb, rbc)

        def wo_tile(st):
            y_ps = pools["wo"].tile([P, D], F32, tag="yps", name=f"yps{st}")
            sl = slice(st * P, (st + 1) * P)
            for n0, nn in ((0, 512), (512, 256)):
                nc.tensor.matmul(
                    y_ps[:, n0 : n0 + nn], OC1[:, sl], wo_b1[:, n0 : n0 + nn],
                    start=True, stop=False,
                )
                nc.tensor.matmul(
                    y_ps[:, n0 : n0 + nn], OC2[:, sl], wo_b2[:, n0 : n0 + nn],
                    start=False, stop=True,
                )
            y_sb = ysb_pool.tile([P, D], F32, tag="ysb", name=f"ysb{st}")
            nc.vector.tensor_copy(out=y_sb, in_=y_ps)
            nc.sync.dma_start(out=y_r[st], in_=y_sb)

        def pair_subsweep(qh, sec, filler):
            """Heads 0,1 over all kt for one 512-q section.  One packed
            [128,1024] score tile per kt ([h0|h1]); exp covers both."""
            q0 = qh * QH + sec * 512
            qsl = slice(q0, q0 + 512)
            ot0 = ot_pool.tile([65, 512], F32, tag="ot", name=f"ot0_{qh}{sec}")
            ot1 = ot_pool.tile([65, 512], F32, tag="ot", name=f"ot1_{qh}{sec}")
            pts = []
            for kt in range(NS):
                ksl = slice(kt * P, (kt + 1) * P)
                sp = s_pool.tile([P, QH], F32, tag="s", name=f"sp{qh}{sec}{kt}")
                nc.tensor.matmul(
                    sp[:, 0:512], KTa[0:DK, ksl], QTa[0:DK, qsl],
                    start=True, stop=True, tile_position=(0, 0),
                )
                nc.tensor.matmul(
                    sp[:, 512:QH], KTa[DK:P, ksl], QTa[DK:P, qsl],
                    start=True, stop=True, tile_position=(64, 0),
                )
                pt = pt_pool.tile([P, QH], BF, tag="pt", name="pt")
                nc.scalar.activation(pt, sp, EXP, bias=0.0, scale=0.125)
                if debug_dump and qh == 0 and sec == 0 and kt == 0:
                    nc.sync.dma_start(out=dbg["dPT0"], in_=pt)
                pts.append(pt)
                # PV chases one kt behind so exp(kt+1) never waits on PE
                if kt >= 1:
                    pv_pair(kt - 1, pts[kt - 1], ot0, ot1)
                    pts[kt - 1] = None
                if filler and kt % 4 == 3:
                    filler(kt // 4)
            pv_pair(NS - 1, pts[NS - 1], ot0, ot1)
            if debug_dump and qh == 0 and sec == 0:
                for nm, ot in (("dOT0", ot0), ("dOT1", ot1)):
                    osb = nrm_pool.tile([65, 512], F32, tag="dbg", name=nm)
                    nc.vector.tensor_copy(out=osb, in_=ot)
                    nc.sync.dma_start(out=dbg[nm], in_=osb)
            normalize(ot0, (slice(0, DK), qsl), False)
            normalize(ot1, (slice(DK, P), qsl), False)

        def pv_pair(kt, pt, ot0, ot1):
            nc.tensor.matmul(
                ot0, V[:, kt, 0:65], pt[:, 0:512],
                start=(kt == 0), stop=(kt == NS - 1),
            )
            nc.tensor.matmul(
                ot1, V[:, kt, 65:130], pt[:, 512:QH],
                start=(kt == 0), stop=(kt == NS - 1),
            )

        def h2_subsweep(qh, sec, filler):
            """Head 2 paired with itself across kt: packed tile is
            [kt_even | kt_odd] for the same 512-q section."""
            q0 = qh * QH + sec * 512
            qsl = slice(q0, q0 + 512)
            ot2 = ot_pool.tile([65, 512], F32, tag="ot", name=f"ot2_{qh}{sec}")
            pts = []
            for kp in range(NS // 2):
                ke, ko = 2 * kp, 2 * kp + 1
                sp = s_pool.tile([P, QH], F32, tag="s", name=f"s2{qh}{sec}{kp}")
                nc.tensor.matmul(
                    sp[:, 0:512], KTb[0:DK, ke * P : (ke + 1) * P], QTb[0:DK, qsl],
                    start=True, stop=True, tile_position=(0, 0),
                )
                nc.tensor.matmul(
                    sp[:, 512:QH], KTb[DK:P, ko * P : (ko + 1) * P], QTb[DK:P, qsl],
                    start=True, stop=True, tile_position=(64, 0),
                )
                pt = pt_pool.tile([P, QH], BF, tag="pt", name="pt")
                nc.scalar.activation(pt, sp, EXP, bias=0.0, scale=0.125)
                if debug_dump and qh == 0 and sec == 0 and kp == 0:
                    nc.sync.dma_start(out=dbg["dPT2"], in_=pt)
                pts.append(pt)
                if kp >= 1:
                    pv2(kp - 1, pts[kp - 1], ot2)
                    pts[kp - 1] = None
                if filler:
                    filler(kp)
            pv2(NS // 2 - 1, pts[NS // 2 - 1], ot2)
            if debug_dump and qh == 0 and sec == 0:
                osb = nrm_pool.tile([65, 512], F32, tag="dbg", name="dOT2")
                nc.vector.tensor_copy(out=osb, in_=ot2)
                nc.sync.dma_start(out=dbg["dOT2"], in_=osb)
            normalize(ot2, qsl, True)

        def pv2(kp, pt, ot2):
            ke, ko = 2 * kp, 2 * kp + 1
            nc.tensor.matmul(
                ot2, V[:, ke, 130:195], pt[:, 0:512],
                start=(kp == 0), stop=False,
            )
            nc.tensor.matmul(
                ot2, V[:, ko, 130:195], pt[:, 512:QH],
                start=False, stop=(kp == NS // 2 - 1),
            )

        # ---------------- phase plumbing ----------------
        # PSUM budget (8 banks): scores 2x[128,1024]=4, ot 2x[65,512]=2,
        # plus 2 for either the small projection pool (q-half 0) or the
        # Wo pool (q-half 1 + tail) -- time-multiplexed via with-scopes.
        pt_pool = ctx.enter_context(tc.tile_pool(name="pt", bufs=4))
        nrm_pool = ctx.enter_context(tc.tile_pool(name="nrm", bufs=2))
        ysb_pool = ctx.enter_context(tc.tile_pool(name="ysb", bufs=2))
        pools = {}

        # K (k<1024) and Q (q<1024) projections via big tiles (PSUM free)
        with tc.tile_pool(name="pj_a", bufs=1, space="PSUM") as pool_a, \
             tc.tile_pool(name="pj_b", bufs=1, space="PSUM") as pool_b:
            qk_proj_big("k", wkb, KTa, KTb, 0, pool_a, pool_b)
            qk_proj_big("q", wqb, QTa, QTb, 0, pool_a, pool_b)

        dup_half(0)

        s_pool = ctx.enter_context(tc.tile_pool(name="s_ps", bufs=2, space="PSUM"))
        ot_pool = ctx.enter_context(tc.tile_pool(name="ot_ps", bufs=2, space="PSUM"))

        # Remaining projections are EMITTED here in dataflow order (reads
        # must follow writes in program order for dependency tracking);
        # the scheduler still overlaps them with the attention sweeps by
        # DMA readiness, using the small 2-bank pj pool.
        def dup_half_k(h):
            sl = slice(h * QH, (h + 1) * QH)
            nc.scalar.dma_start(out=KTb[DK:P, sl], in_=KTb[0:DK, sl])

        def dup_half_q(h):
            sl = slice(h * QH, (h + 1) * QH)
            nc.scalar.dma_start(out=QTb[DK:P, sl], in_=QTb[0:DK, sl])

        with tc.tile_pool(name="pj_s", bufs=1, space="PSUM") as pjp:
            pools["pj"] = pjp
            qk_proj_small("k", wkb, KTa, KTb, 1, pjp)
            dup_half_k(1)
            v_proj_half(0, pjp)
            v_proj_half(1, pjp)
            qk_proj_small("q", wqb, QTa, QTb, 1, pjp)
            dup_half_q(1)
            pair_subsweep(0, 0, None)
            pair_subsweep(0, 1, None)
            h2_subsweep(0, 0, None)
            h2_subsweep(0, 1, None)

        # ---- q-half 1: pair subsweeps with Wo(half0) interleaved ----
        with tc.tile_pool(name="wo_ps", bufs=1, space="PSUM") as wop:
            pools["wo"] = wop
            pair_subsweep(1, 0, lambda i: wo_tile(i) if i < 4 else None)
            pair_subsweep(1, 1, lambda i: wo_tile(4 + i) if i < 4 else None)

            # h2 of half 1; remaining wo tiles interleave after normalizes
            # complete (s-tiles 8..11 during sec B, 12..15 at tail)
            h2_subsweep(1, 0, None)

            def fill_h2_1B(i):
                if i < 4:
                    wo_tile(8 + i)

            h2_subsweep(1, 1, fill_h2_1B)
            for st in range(12, NS):
                wo_tile(st)

        if debug_dump:
            nc.sync.dma_start(out=dbg["dQTa"], in_=QTa)
            nc.sync.dma_start(out=dbg["dKTa"], in_=KTa)
            nc.sync.dma_start(out=dbg["dQTb"], in_=QTb)
            nc.sync.dma_start(out=dbg["dKTb"], in_=KTb)
            nc.sync.dma_start(out=dbg["dV"], in_=V.rearrange("p a b -> p (a b)"))
            nc.sync.dma_start(out=dbg["dOC1"], in_=OC1)
            nc.sync.dma_start(out=dbg["dOC2"], in_=OC2)

    nc.compile()
    return nc


def kernel(query, key, value, Wq, bq, Wk, bk, Wv, bv, Wo, bo, **_ignored):
    from concourse.bass_utils import run_bass_kernel_spmd

    query = np.asarray(query, dtype=np.float32)
    key = np.asarray(key, dtype=np.float32)
    value = np.asarray(value, dtype=np.float32)
    Wq = np.asarray(Wq, dtype=np.float32)
    Wk = np.asarray(Wk, dtype=np.float32)
    Wv = np.asarray(Wv, dtype=np.float32)
    Wo = np.asarray(Wo, dtype=np.float32)
    bq = np.asarray(bq, dtype=np.float32)
    bk = np.asarray(bk, dtype=np.float32)
    bv = np.asarray(bv, dtype=np.float32)
    bo = np.asarray(bo, dtype=np.float32)

    use_bias_qkv = bool(np.any(bq) or np.any(bk) or np.any(bv))
    if "nc" not in _CACHE or _CACHE.get("bias") != use_bias_qkv:
        _CACHE["nc"] = _build_nc(use_bias_qkv)
        _CACHE["bias"] = use_bias_qkv
    nc = _CACHE["nc"]

    xT = {b: {} for b in range(B)}
    for b in range(B):
        xT[b]["q"] = np.ascontiguousarray(query[b].T)
        xT[b]["k"] = np.ascontiguousarray(key[b].T)
        xT[b]["v"] = np.ascontiguousarray(value[b].T)

    in_maps = []
    for c in range(8):
        b, g = divmod(c, 4)
        hs = slice(g * HD, (g + 1) * HD)
        wv_p = np.zeros((D, VP), dtype=np.float32)
        wv_p[:, 0:HD] = Wv[:, hs]
        in_maps.append({
            "xqT": xT[b]["q"],
            "xkT": xT[b]["k"],
            "xvT": xT[b]["v"],
            "wq": np.ascontiguousarray(Wq[:, hs]),
            "wk": np.ascontiguousarray(Wk[:, hs]),
            "wv": wv_p,
            "wo": np.ascontiguousarray(Wo[hs, :]),
            "bqkv": np.ascontiguousarray(
                np.stack([bq[hs], bk[hs], bv[hs]]).astype(np.float32)
            ),
        })

    res = run_bass_kernel_spmd(nc, in_maps, core_ids=list(range(8)), **_CACHE.get("run_kwargs", {}))
    _CACHE["last_result"] = res

    out = np.empty((B, S, D), dtype=np.float32)
    for b in range(B):
        acc = res.results[4 * b]["y"].astype(np.float32).copy()
        for g in range(1, 4):
            acc += res.results[4 * b + g]["y"]
        out[b] = acc + bo[None, :]
    return out
